# revision 84
# baseline (speedup 1.0000x reference)
"""Linear attention (silu+1 feature map) MultiHeadAttention kernel for 8x TRN2.

Sharding: data-parallel over batch (B=8 -> 1 batch element per NeuronCore).

All four big GEMMs + the kv accumulation run in fp8 e4m3 with
perf_mode=DoubleRow (2 contraction rows per PE cell -> K=256 per matmul).
Empirical end-to-end rel err of the full fp8 pipeline vs the fp32 reference
is ~4.6e-3 (fp8_exact.py), well within tolerance, because the "+1" in the
silu+1 feature map makes the output dominated by coherent sums.

The +1 itself never materializes on device; it is folded algebraically:
  phi_k "+1":  kv_h += sum_t v_h[t,:]  == rank-8 S-init of each kv PSUM bank
               (S = xsum @ Wv.T + T*bv, host-exact, one K=8 bf16 matmul/bank)
  phi_q "+1":  y += colsum(M)          == folded into the y-drain bias
               (colsum computed on device from fp8 M via ones-matmuls)

kv lives in TWO PSUM banks (heads 0-7 / 8-15) with every head's [64,64]
kvT block at PSUM rows 0:64, because DoubleRow matmuls cannot write dst
base partition 64 (ISA s3d3_mm_valid_dst_partition). Schedule: pairs 1-2
interleave one kv sub-tile between q oc boundaries so ACT drain demand
never bursts above the PE issue rate; pair 0 stays sequential (weights
still in flight on the serialized DMA timeline); the last pair hides the
kv flush + repack + M stage + colsum under its q section via per-oc hooks.

Per-core math (T=4096, D=1024, H=16, Dh=64), fp32 PSUM accumulation:
  phase 1 (stream token tiles):
    qT[o,t]   = WqT.T @ xT          (DoubleRow fp8, feature-major)
    phi_qT    = silu(s*qT + s*bq)   (ACT -> fp8, NO +1)
    k[t,e]    = xT.T @ WkT          (DoubleRow fp8, token-major)
    v[t,e]    = xT.T @ WvT          (DoubleRow fp8)
    phi_k     = silu(s*k)           (ACT -> fp8, NO +1)
    v8        = v + bv              (DVE -> fp8)
    kv_h     += v8_h.T @ phi_k_h    (DoubleRow fp8 over 256-token blocks,
                                     on top of the S-init)
  M stage (hidden under the last q section):
    M_h[d,o]  = kv_h.T @ WoT_h      (bf16)  -> m8 fp8 [128,DC,D]
    colsum[o] = ones.T @ m8         (64 tiny matmuls into the freed kv bank)
    bo2       = bo + colsum
  phase 2:
    yT[o,t]   = M.T @ phi_qT + bo2  (DoubleRow fp8) -> bf16 out

Host side: casts x.T / weights to e4m3, computes S exactly, casts y back
to fp32.
"""

import numpy as np
import ml_dtypes

B, T, D = 8, 4096, 1024
H, DH = 16, 64
SCALE = float(DH ** -0.25)
NCORES = 8
P = 128
DC = D // P          # 8 feature chunks
NG = DC // 2         # 4 DoubleRow contraction groups (K=256 each)
TT = 512             # token tile (phase 1)
NTT = T // TT        # 8 token tiles
NSUB = TT // P       # 4 sub-tiles of 128 tokens
NBLK = T // 256      # 16 256-token kv blocks

_BF16 = ml_dtypes.bfloat16
_E4 = ml_dtypes.float8_e4m3

_CACHE = {}


def _split_multi_waits(nc):
    """walrus in this container only encodes ONE sync-wait command per
    instruction. Hoist extra waits onto injected same-engine NOPs placed
    immediately before the instruction (program order on the engine queue
    makes this semantically identical)."""
    import concourse.mybir as mybir

    n_split = 0
    for fn in nc.m.functions:
        for bb in fn.blocks:
            new = []
            changed = False
            for inst in bb.instructions:
                si = inst.sync_info
                waits = list(si.on_wait) if si is not None else []
                if len(waits) > 1:
                    changed = True
                    # satisfy-order heuristic: DMA-queue sems complete last
                    # (the final y transfers), so put them at the END of the
                    # serial NoOp chain — the other NoOps' decode time then
                    # hides under the DMA wait instead of after it
                    waits.sort(
                        key=lambda w: (
                            "DMA" in (w.ant_name or "")
                            or "q" == (w.ant_name or "")[:1]
                        )
                    )
                    for j, w in enumerate(waits[:-1]):
                        nop = mybir.InstNoOp(
                            name=f"{inst.name}-sw{j}", ins=[], outs=[]
                        )
                        nop.engine = inst.engine
                        nop.sync_info = mybir.SyncInfo(
                            on_wait=[w], on_update=[]
                        )
                        new.append(nop)
                        n_split += 1
                    inst.sync_info = mybir.SyncInfo(
                        on_wait=[waits[-1]], on_update=list(si.on_update)
                    )
                new.append(inst)
            if changed:
                bb.instructions = new
    return n_split


def _build_program(debug=False):
    import concourse.bass as bass
    import concourse.mybir as mybir
    from concourse.tile import TileContext, add_dep_helper

    dt = mybir.dt
    AF = mybir.ActivationFunctionType
    DR = mybir.MatmulPerfMode.DoubleRow

    nc = bass.Bass()

    xT_d = nc.dram_tensor("xT8", [D, T], dt.float8e4, kind="ExternalInput")
    wq_d = nc.dram_tensor("wq8", [D, D], dt.float8e4, kind="ExternalInput")
    wk_d = nc.dram_tensor("wk8", [D, D], dt.float8e4, kind="ExternalInput")
    wv_d = nc.dram_tensor("wv8", [D, D], dt.float8e4, kind="ExternalInput")
    wo_d = nc.dram_tensor("wo", [D, D], dt.bfloat16, kind="ExternalInput")
    bqs_d = nc.dram_tensor("bqs", [P, DC], dt.float32, kind="ExternalInput")
    bos_d = nc.dram_tensor("bos", [P, DC], dt.float32, kind="ExternalInput")
    bvb_d = nc.dram_tensor("bvb", [P, D], dt.float32, kind="ExternalInput")
    sv_d = nc.dram_tensor("sv", [DC, P], dt.bfloat16, kind="ExternalInput")
    ev_d = nc.dram_tensor("ev", [DC, 512], dt.bfloat16, kind="ExternalInput")
    yT_d = nc.dram_tensor("yT", [D, T], dt.bfloat16, kind="ExternalOutput")
    if debug:
        phiq_d = nc.dram_tensor("phiq_dump", [P, DC, T], dt.float8e4, kind="ExternalOutput")
        kv_d = nc.dram_tensor("kv_dump", [P, 512], dt.float32, kind="ExternalOutput")
        m_d = nc.dram_tensor("m_dump", [P, DC, D], dt.float8e4, kind="ExternalOutput")
        bo2_d = nc.dram_tensor("bo2_dump", [P, DC], dt.float32, kind="ExternalOutput")

    with TileContext(nc) as tc:
        with (
            tc.tile_pool(name="weights", bufs=1) as wpool,
            tc.tile_pool(name="phiq", bufs=1) as qpool,
            tc.tile_pool(name="msb", bufs=1) as mpool,
            tc.tile_pool(name="xin", bufs=4) as xpool,
            tc.tile_pool(name="kvtiles", bufs=8) as kvpool,
            tc.tile_pool(name="yout", bufs=4) as ypool,
        ):
            # ---- weight / const preload ----
            wq_sb = wpool.tile([P, DC, D], dt.float8e4, tag="wq")
            wk_sb = wpool.tile([P, DC, D], dt.float8e4, tag="wk")
            wv_sb = wpool.tile([P, DC, D], dt.float8e4, tag="wv")
            wo_sb = wpool.tile([P, DC, D], dt.bfloat16, tag="wo")
            bq_sb = wpool.tile([P, DC], dt.float32, tag="bq")
            bos_sb = wpool.tile([P, DC], dt.float32, tag="bos")
            bo2_sb = wpool.tile([P, DC], dt.float32, tag="bo2")
            bv_sb = wpool.tile([P, D], dt.float32, tag="bv")
            sv_sb = wpool.tile([DC, P], dt.bfloat16, tag="sv")
            ev_sb = wpool.tile([DC, 512], dt.bfloat16, tag="ev")
            # The cost model serializes all DMA transfers in arrival order, so
            # issue strictly in first-needed order on the sync queue: tiny
            # consts, wq half, first two x tiles, rest of the weights. Later
            # x tiles go on the gpsimd queue, dep-pinned so they can't race
            # ahead of the weight transfers.
            wq_r = wq_d.rearrange("(c p) o -> p c o", p=P)
            xT_r = xT_d.rearrange("(c p) t -> p c t", p=P)
            # strict first-needed order on one queue: the cost model
            # serializes all DMA transfers, so ordering IS the schedule
            xt_pre = []
            for half in range(2):
                xt0 = xpool.tile([P, DC, TT], dt.float8e4, tag="xt", name=f"xtpre{half}")
                if half == 0:
                    nc.sync.dma_start(xt0[:], xT_r[:, :, 0:TT])
                    # oc0 only needs wq columns 0:128 — tiny first slice so
                    # the first q matmul fires as early as possible
                    nc.sync.dma_start(wq_sb[:, :, 0:128], wq_r[:, :, 0:128])
                else:
                    nc.sync.dma_start(xt0[:], xT_r[:, :, TT : 2 * TT])
                xt_pre.append(xt0)
            nc.sync.dma_start(sv_sb[:], sv_d[:])
            nc.sync.dma_start(bq_sb[:], bqs_d[:])
            nc.sync.dma_start(ev_sb[:], ev_d[:])
            nc.sync.dma_start(wq_sb[:, :, 128:512], wq_r[:, :, 128:512])
            nc.sync.dma_start(wq_sb[:, :, 512:1024], wq_r[:, :, 512:1024])
            wk_r = wk_d.rearrange("(c p) o -> p c o", p=P)
            wv_r = wv_d.rearrange("(c p) o -> p c o", p=P)
            nc.sync.dma_start(wk_sb[:, :, 0:512], wk_r[:, :, 0:512])
            nc.sync.dma_start(wk_sb[:, :, 512:1024], wk_r[:, :, 512:1024])
            nc.sync.dma_start(wv_sb[:, :, 0:512], wv_r[:, :, 0:512])
            nc.sync.dma_start(wv_sb[:, :, 512:1024], wv_r[:, :, 512:1024])
            nc.sync.dma_start(bv_sb[:], bvb_d[:])
            # wo + bos are deferred into the pair loop (pair 3) so the
            # streaming x tiles win the serialized DMA resource first

            phi_q = qpool.tile([P, DC, T], dt.float8e4, tag="phiq")
            m8 = mpool.tile([P, DC, D], dt.float8e4, tag="m8")
            kv_chunks = []
            for c in range(DC):
                kvc = mpool.tile(
                    [P, P], dt.bfloat16, tag=f"kvsb{c}", name=f"kvsb{c}"
                )
                kv_chunks.append(kvc)

            zz = wpool.tile([1, 640], dt.bfloat16, tag="zz")
            nc.vector.memset(zz[:], 0.0)
            ones8 = wpool.tile([P, 1], dt.float8e4, tag="ones8")
            nc.vector.memset(ones8[:], 1.0)
            # kv chunk off-diagonal blocks must be zero (block-diag repack)
            for c in range(DC):
                nc.vector.memset(kv_chunks[c][:], 0.0)

            with tc.tile_pool(name="ps_kv", bufs=1, space="PSUM") as pkv_pool:
                # two kv banks: heads 0-7 in A, 8-15 in B, every head's
                # [64,64] kvT block at PSUM rows 0:64 so ALL kv accumulation
                # matmuls are DoubleRow-legal (dst base partition 0)
                kv_psA = pkv_pool.tile([P, 512], dt.float32, tag="kvaccA")
                kv_psB = pkv_pool.tile([P, 512], dt.float32, tag="kvaccB")
                # warmup matmuls filling the startup DMA shadow: they keep the
                # PE p-state warm so the first real matmuls run at full clock
                # the moment weights land. The kv bank content they write is
                # dead — the S-init matmul below re-initializes it.
                nc.tensor.matmul(
                    kv_psA[:], lhsT=zz[:1, :P], rhs=zz[:1, P : P + 512],
                    start=True, stop=True, skip_group_check=True,
                )
                for w in range(38):
                    nc.tensor.matmul(
                        kv_psA[:, 0:128], lhsT=zz[:1, :P], rhs=zz[:1, P : P + 128],
                        start=True, stop=True, skip_group_check=True,
                    )

                kv_pend = [None]

                def _emit_kv(pending, last):
                    # one DoubleRow matmul per head per 256-token block, all
                    # at dst base partition 0 (ISA requirement for DR)
                    phik_p, vsb_p = pending
                    for h in range(H):
                        bank = kv_psA if h < 8 else kv_psB
                        c0 = (h % 8) * 64
                        nc.tensor.matmul(
                            bank[0:64, c0 : c0 + 64],
                            lhsT=vsb_p[:, :, h * 64 : (h + 1) * 64],
                            rhs=phik_p[:, :, h * 64 : (h + 1) * 64],
                            start=False,
                            stop=last and h == H - 1,
                            perf_mode=DR,
                            skip_group_check=True,
                        )

                with tc.tile_pool(name="ps_q", bufs=6, space="PSUM") as pq_pool:
                  if True:
                      pkvp_pool = pq_pool
                      # ---- q projection (feature-major out, DoubleRow).
                      # first_split runs tile A before tile B (pair 0: B's DMA
                      # still in flight), bridging with PE warmup dummies.
                      def _q_section(pair, xts, first_split, post_oc=None):
                          def _drain(oc, half, psx):
                              tt = pair * 2 + half
                              pq_slice = phi_q[:, oc, tt * TT : (tt + 1) * TT]
                              nc.scalar.activation(
                                  pq_slice, psx[:], AF.Silu,
                                  bias=bq_sb[:, oc : oc + 1], scale=SCALE,
                              )

                          for oc in range(DC):
                              psA = pq_pool.tile([P, TT], dt.float32, tag="ps")
                              psB = pq_pool.tile([P, TT], dt.float32, tag="ps")
                              if first_split and oc == 0:
                                  last_a = None
                                  for g in range(NG):
                                      last_a = nc.tensor.matmul(
                                          psA[:],
                                          lhsT=wq_sb[:, 2 * g : 2 * g + 2, oc * P : (oc + 1) * P],
                                          rhs=xts[0][:, 2 * g : 2 * g + 2, :],
                                          start=(g == 0),
                                          stop=(g == NG - 1),
                                          perf_mode=DR,
                                      )
                                  # bridge the B-tile DMA wait with warmup
                                  # zero-rewrites of the (still dead) kv bank;
                                  # dep-pinned after the A matmuls so the
                                  # scheduler cannot hoist them earlier
                                  for w in range(14):
                                      dmy = nc.tensor.matmul(
                                          kv_psA[:, 0:128],
                                          lhsT=zz[:1, :P],
                                          rhs=zz[:1, P : P + 128],
                                          start=True, stop=True,
                                          skip_group_check=True,
                                      )
                                      add_dep_helper(
                                          dmy.ins, last_a.ins, sync=False,
                                          reason="bridge dummies after A matmuls",
                                      )
                                  # S-init: bank[r, c] = S[hbase + (c//64)*64 + r]
                                  # (the phi_k "+1" fold; kv blocks hold
                                  # kv_h TRANSPOSED so S varies along rows).
                                  # Must be the LAST start=True write to each
                                  # kv bank.
                                  for bank, c0_ in ((kv_psA, 0), (kv_psB, 64)):
                                      dmy = nc.tensor.matmul(
                                          bank[0:64, :],
                                          lhsT=sv_sb[:, c0_ : c0_ + 64],
                                          rhs=ev_sb[:, :],
                                          start=True, stop=True,
                                          skip_group_check=True,
                                      )
                                      add_dep_helper(
                                          dmy.ins, last_a.ins, sync=False,
                                          reason="S-init after A matmuls",
                                      )
                                  for g in range(NG):
                                      nc.tensor.matmul(
                                          psB[:],
                                          lhsT=wq_sb[:, 2 * g : 2 * g + 2, oc * P : (oc + 1) * P],
                                          rhs=xts[1][:, 2 * g : 2 * g + 2, :],
                                          start=(g == 0),
                                          stop=(g == NG - 1),
                                          perf_mode=DR,
                                      )
                              else:
                                  for g in range(NG):
                                      nc.tensor.matmul(
                                          psA[:],
                                          lhsT=wq_sb[:, 2 * g : 2 * g + 2, oc * P : (oc + 1) * P],
                                          rhs=xts[0][:, 2 * g : 2 * g + 2, :],
                                          start=(g == 0),
                                          stop=(g == NG - 1),
                                          perf_mode=DR,
                                      )
                                      nc.tensor.matmul(
                                          psB[:],
                                          lhsT=wq_sb[:, 2 * g : 2 * g + 2, oc * P : (oc + 1) * P],
                                          rhs=xts[1][:, 2 * g : 2 * g + 2, :],
                                          start=(g == 0),
                                          stop=(g == NG - 1),
                                          perf_mode=DR,
                                      )
                              _drain(oc, 0, psA)
                              _drain(oc, 1, psB)
                              if post_oc is not None and oc in post_oc:
                                  post_oc[oc]()

                      # ---- k,v projections (token-major, DoubleRow) + kv
                      # accumulation over 256-token blocks. kv matmuls for a
                      # block are emitted one block LATE so PE never waits on
                      # the silu/+bv drains. ----
                      blk_state = {}

                      def _kv_sub(pair, xts, half, sub):
                          tt = pair * 2 + half
                          xt = xts[half]
                          blk = (tt * NSUB + sub) // 2
                          i = sub % 2
                          if i == 0:
                              phik_b = kvpool.tile([P, 2, D], dt.float8e4, tag="phik")
                              v_b = kvpool.tile([P, 2, D], dt.float8e4, tag="vsb")
                              blk_state[blk] = (phik_b, v_b)
                          else:
                              phik_b, v_b = blk_state[blk]
                          # 4 one-bank psum tiles per sub so each half-drain
                          # releases its bank early
                          pk0 = pkvp_pool.tile([P, 512], dt.float32, tag="ps")
                          pk1 = pkvp_pool.tile([P, 512], dt.float32, tag="ps")
                          pv0 = pkvp_pool.tile([P, 512], dt.float32, tag="ps")
                          pv1 = pkvp_pool.tile([P, 512], dt.float32, tag="ps")
                          kvps = (pk0, pk1, pv0, pv1)
                          xs = xt[:, :, sub * P : (sub + 1) * P]
                          for g in range(NG):
                              for n, ps in enumerate(kvps):
                                  w_sb = wk_sb if n < 2 else wv_sb
                                  col = (n % 2) * 512
                                  nc.tensor.matmul(
                                      ps[:],
                                      lhsT=xs[:, 2 * g : 2 * g + 2, :],
                                      rhs=w_sb[:, 2 * g : 2 * g + 2, col : col + 512],
                                      start=(g == 0),
                                      stop=(g == NG - 1),
                                      perf_mode=DR,
                                  )
                          nc.scalar.activation(
                              phik_b[:, i, 0:512], pk0[:], AF.Silu, scale=SCALE
                          )
                          nc.vector.tensor_add(
                              v_b[:, i, 0:512], pv0[:], bv_sb[:, 0:512]
                          )
                          nc.scalar.activation(
                              phik_b[:, i, 512:1024], pk1[:], AF.Silu, scale=SCALE
                          )
                          nc.vector.tensor_add(
                              v_b[:, i, 512:1024], pv1[:], bv_sb[:, 512:1024]
                          )
                          if i == 1:
                              if kv_pend[0] is not None:
                                  _emit_kv(kv_pend[0], False)
                              kv_pend[0] = blk_state.pop(blk)

                      def _kvproj_section(pair, xts):
                          for half in range(2):
                              for sub in range(NSUB):
                                  _kv_sub(pair, xts, half, sub)

                      for pair in range(NTT // 2):
                          if pair == 0:
                              xts = xt_pre
                          else:
                              xts = []
                              for half in range(2):
                                  tt = pair * 2 + half
                                  xt = xpool.tile([P, DC, TT], dt.float8e4, tag="xt")
                                  nc.sync.dma_start(
                                      xt[:], xT_r[:, :, tt * TT : (tt + 1) * TT]
                                  )
                                  xts.append(xt)
                              if pair == NTT // 2 - 1:
                                  nc.sync.dma_start(
                                      wo_sb[:],
                                      wo_d.rearrange("(c p) o -> p c o", p=P),
                                  )
                                  nc.sync.dma_start(bos_sb[:], bos_d[:])

                          if pair == NTT // 2 - 1:
                              # last pair: kvproj first, then q. The kv flush,
                              # repack copies, M matmuls, m8 drains, colsum and
                              # bo2 are spread across the q chunk boundaries,
                              # hidden under the q matmul stream. (Not
                              # interleaved: kv must finish before the flush.)
                              _kvproj_section(pair, xts)

                              def _hook_flush():
                                  _emit_kv(kv_pend[0], True)
                                  kv_pend[0] = None

                              def _repack(c):
                                  # repack chunk c, alternating engines
                                  if c % 2 == 0:
                                      use_dve = True
                                  else:
                                      use_dve = False
                                  for hh, rr in ((2 * c, 0), (2 * c + 1, 64)):
                                      bank = kv_psA if hh < 8 else kv_psB
                                      src_c = (hh % 8) * 64
                                      dst = kv_chunks[c][rr : rr + 64, rr : rr + 64]
                                      if use_dve:
                                          nc.vector.tensor_copy(
                                              out=dst,
                                              in_=bank[0:64, src_c : src_c + 64],
                                          )
                                      else:
                                          nc.scalar.copy(
                                              out=dst,
                                              in_=bank[0:64, src_c : src_c + 64],
                                          )

                              def _mk_hook_m(c):
                                  def _hook():
                                      if c == 0:
                                          for cc in range(DC):
                                              _repack(cc)
                                      for n in range(2):
                                          pm = pkvp_pool.tile(
                                              [P, 512], dt.float32, tag="ps"
                                          )
                                          nc.tensor.matmul(
                                              pm[:],
                                              lhsT=kv_chunks[c][:],
                                              rhs=wo_sb[:, c, n * 512 : (n + 1) * 512],
                                              start=True,
                                              stop=True,
                                          )
                                          # DVE-only drains: ACT is the
                                          # scarce engine in the last-pair
                                          # q section
                                          dst = m8[:, c, n * 512 : (n + 1) * 512]
                                          nc.vector.tensor_copy(
                                              out=dst, in_=pm[:]
                                          )
                                  return _hook

                              def _colsum_chunks(cs, first, last):
                                  # colsum[o] = sum_f m8[f, o] accumulated in
                                  # the freed kv bank, cols 0:DC, one column
                                  # per o-chunk — spread incrementally so each
                                  # piece waits only on its own m8 chunks.
                                  if first:
                                      nc.tensor.matmul(
                                          kv_psA[:, 0:DC], lhsT=zz[:1, :P],
                                          rhs=zz[:1, P : P + DC],
                                          start=True, stop=True,
                                          skip_group_check=True,
                                      )
                                  for c in cs:
                                      for oc2 in range(DC):
                                          nc.tensor.matmul(
                                              kv_psA[:, oc2 : oc2 + 1],
                                              lhsT=m8[:, c, oc2 * P : (oc2 + 1) * P],
                                              rhs=ones8[:, 0:1],
                                              start=False,
                                              stop=(last and c == cs[-1] and oc2 == DC - 1),
                                              skip_group_check=True,
                                          )
                                  if last:
                                      nc.vector.tensor_add(
                                          bo2_sb[:], kv_psA[:, 0:DC], bos_sb[:]
                                      )

                              # hook layout: flush+repack+M(c0) at oc0, then
                              # one M chunk per oc, with each chunk's colsum
                              # two ocs behind its m8 drain; the last two
                              # colsum pieces + bo2 land right after the
                              # section (phase 2's first drain ~1us in).
                              def _mk_hook(oc):
                                  def _hook():
                                      if oc == 0:
                                          _hook_flush()
                                      _mk_hook_m(oc)()
                                      if oc >= 2:
                                          _colsum_chunks(
                                              [oc - 2], first=(oc == 2),
                                              last=False,
                                          )
                                  return _hook

                              hooks = {oc: _mk_hook(oc) for oc in range(DC)}
                              _q_section(pair, xts, False, hooks)
                              _colsum_chunks([DC - 2], first=False, last=False)
                              _colsum_chunks([DC - 1], first=False, last=True)
                          else:
                              # interleave kv subs between q oc boundaries so
                              # ACT drain demand never bursts above the PE
                              # rate. Pair 0 front-loads 4 q ocs (the kv
                              # weights are still in flight on the serialized
                              # DMA timeline), then 2 subs per oc.
                              def _mk_kv_hook(subs, pair=pair, xts=xts):
                                  def _h():
                                      for s in subs:
                                          _kv_sub(pair, xts, s // NSUB, s % NSUB)
                                  return _h

                              if pair == 0:
                                  hooks = {
                                      3 + j: _mk_kv_hook([2 * j, 2 * j + 1])
                                      for j in range(4)
                                  }
                              else:
                                  hooks = {
                                      i: _mk_kv_hook([i]) for i in range(DC)
                                  }
                              _q_section(pair, xts, pair == 0, hooks)

                      if kv_pend[0] is not None:
                          _emit_kv(kv_pend[0], True)
                          kv_pend[0] = None

                if debug:
                    kvf = mpool.tile([P, 512], dt.float32, tag="kvdump")
                    nc.vector.tensor_copy(out=kvf[:], in_=kv_psA[:])
                    nc.sync.dma_start(kv_d[:], kvf[:])
                    bo2f = mpool.tile([P, DC], dt.float32, tag="bo2dump")
                    nc.vector.tensor_copy(out=bo2f[:], in_=bo2_sb[:])
                    nc.sync.dma_start(bo2_d[:], bo2f[:])
            # ---- phase 2: yT = M.T @ phi_q + bo2 (DoubleRow) ----
            # Per oc: 4 qb blocks of [128,1024] psum; drains split
            # ACT/DVE per 512 half into a [P, T] bf16 staging tile,
            # DMA'd out in 2048-token halves on sync/gpsimd queues.
            # (kv pool closed above -> 8 banks free for bufs=4.)
            with tc.tile_pool(name="ps_y", bufs=4, space="PSUM") as py_pool:
                for oc in range(DC):
                    ys = ypool.tile([P, T], dt.bfloat16, tag="ys")
                    for qb in range(4):
                        py = py_pool.tile([P, 1024], dt.float32, tag="py")
                        for i in range(2):
                            for g in range(NG):
                                nc.tensor.matmul(
                                    py[:, i * 512 : (i + 1) * 512],
                                    lhsT=m8[:, 2 * g : 2 * g + 2, oc * P : (oc + 1) * P],
                                    rhs=phi_q[
                                        :, 2 * g : 2 * g + 2,
                                        qb * 1024 + i * 512 : qb * 1024 + (i + 1) * 512,
                                    ],
                                    start=(g == 0),
                                    stop=(g == NG - 1),
                                    perf_mode=DR,
                                )
                        # one full-width drain per qb, alternating engines:
                        # halves the instruction+semaphore count vs split
                        # drains, keeping both engines under the PE rate.
                        # Very last block: two half drains on both engines so
                        # the kernel tail is one small piece.
                        ys_sl = ys[:, qb * 1024 : (qb + 1) * 1024]
                        if oc == DC - 1 and qb == 3:
                            nc.scalar.activation(
                                ys[:, qb * 1024 : qb * 1024 + 512],
                                py[:, 0:512], AF.Identity,
                                bias=bo2_sb[:, oc : oc + 1], scale=1.0,
                            )
                            nc.vector.tensor_scalar_add(
                                ys[:, qb * 1024 + 512 : (qb + 1) * 1024],
                                py[:, 512:1024], bo2_sb[:, oc : oc + 1],
                            )
                        elif qb % 2 == 1:
                            nc.scalar.activation(
                                ys_sl, py[:], AF.Identity,
                                bias=bo2_sb[:, oc : oc + 1], scale=1.0,
                            )
                        else:
                            nc.vector.tensor_scalar_add(
                                ys_sl, py[:], bo2_sb[:, oc : oc + 1],
                            )
                        # per-qb 1024-token DMAs spread evenly over both
                        # queues so no end-of-kernel burst; the final qb goes
                        # out as two 512 halves so the last transfer is small
                        if oc == DC - 1 and qb == 3:
                            nc.sync.dma_start(
                                yT_d[oc * P : (oc + 1) * P, 3072:3584],
                                ys[:, 3072:3584],
                            )
                            nc.gpsimd.dma_start(
                                yT_d[oc * P : (oc + 1) * P, 3584:4096],
                                ys[:, 3584:4096],
                            )
                        else:
                            q_eng = nc.sync if qb % 2 == 0 else nc.gpsimd
                            q_eng.dma_start(
                                yT_d[oc * P : (oc + 1) * P,
                                     qb * 1024 : (qb + 1) * 1024],
                                ys[:, qb * 1024 : (qb + 1) * 1024],
                            )

            if debug:
                nc.sync.dma_start(phiq_d[:], phi_q[:])
                nc.sync.dma_start(m_d[:], m8[:])
    _split_multi_waits(nc)
    return nc


def _get_program(debug=False):
    key = ("nc", debug)
    if key not in _CACHE:
        _CACHE[key] = _build_program(debug)
    return _CACHE[key]


def _prep_shared(Wq, bq, Wk, Wv, bv, Wo, bo):
    shared = {
        "wq8": np.ascontiguousarray(Wq.T).astype(_E4),
        "wk8": np.ascontiguousarray(Wk.T).astype(_E4),
        "wv8": np.ascontiguousarray(Wv.T).astype(_E4),
        "wo": np.ascontiguousarray(Wo.T).astype(_BF16),
        "bqs": np.ascontiguousarray(
            (SCALE * bq).astype(np.float32).reshape(DC, P).T
        ),
        "bos": np.ascontiguousarray(bo.astype(np.float32).reshape(DC, P).T),
        "bvb": np.ascontiguousarray(
            np.broadcast_to(bv.astype(np.float32), (P, D))
        ),
        "ev": np.ascontiguousarray(
            np.repeat(np.eye(DC, dtype=np.float32), 64, axis=1)
        ).astype(_BF16),
    }
    return shared


def _pack_sv(S):
    # bank[r, c] inits to S[hbase*64 + (c//64)*64 + r]: per bank a rank-8
    # matmul with U = S-slice.reshape(8, 64) stationary (column-packed so
    # both banks' U slices sit at base partition 0) and the 64-wide column
    # indicator moving (built in _prep_shared)
    s2 = np.concatenate(
        [S[:512].reshape(DC, 64), S[512:].reshape(DC, 64)], axis=1
    )
    return np.ascontiguousarray(s2).astype(_BF16)


def _run(in_maps, trace=False, debug=False, **kw):
    from concourse.bass_utils import run_bass_kernel_spmd

    nc = _get_program(debug)
    return run_bass_kernel_spmd(nc, in_maps, list(range(NCORES)), trace=trace, **kw)


def kernel(x, Wq, bq, Wk, Wv, bv, Wo, bo):
    x = np.asarray(x, dtype=np.float32)
    assert x.shape == (B, T, D), x.shape
    Wv32 = np.asarray(Wv, np.float32)
    bv32 = np.asarray(bv, np.float32)
    shared = _prep_shared(
        np.asarray(Wq, np.float32), np.asarray(bq, np.float32),
        np.asarray(Wk, np.float32), Wv32,
        bv32, np.asarray(Wo, np.float32),
        np.asarray(bo, np.float32),
    )
    xsum = x.sum(axis=1)  # [B, D]
    in_maps = []
    for b in range(B):
        m = dict(shared)
        m["xT8"] = np.ascontiguousarray(x[b].T).astype(_E4)
        S = xsum[b] @ Wv32.T + T * bv32
        m["sv"] = _pack_sv(S)
        in_maps.append(m)

    res = _run(in_maps)
    out = np.empty((B, T, D), np.float32)
    for b in range(B):
        out[b] = res.results[b]["yT"].T.astype(np.float32)
    return out


# revision 85
# speedup vs baseline: 1.0004x; 1.0004x over previous
"""Linear attention (silu+1 feature map) MultiHeadAttention kernel for 8x TRN2.

Sharding: data-parallel over batch (B=8 -> 1 batch element per NeuronCore).

All four big GEMMs + the kv accumulation run in fp8 e4m3 with
perf_mode=DoubleRow (2 contraction rows per PE cell -> K=256 per matmul).
Empirical end-to-end rel err of the full fp8 pipeline vs the fp32 reference
is ~4.6e-3 (fp8_exact.py), well within tolerance, because the "+1" in the
silu+1 feature map makes the output dominated by coherent sums.

The +1 itself never materializes on device; it is folded algebraically:
  phi_k "+1":  kv_h += sum_t v_h[t,:]  == rank-8 S-init of each kv PSUM bank
               (S = xsum @ Wv.T + T*bv, host-exact, one K=8 bf16 matmul/bank)
  phi_q "+1":  y += colsum(M)          == folded into the y-drain bias
               (colsum computed on device from fp8 M via ones-matmuls)

kv lives in TWO PSUM banks (heads 0-7 / 8-15) with every head's [64,64]
kvT block at PSUM rows 0:64, because DoubleRow matmuls cannot write dst
base partition 64 (ISA s3d3_mm_valid_dst_partition). Schedule: pairs 1-2
interleave one kv sub-tile between q oc boundaries so ACT drain demand
never bursts above the PE issue rate; pair 0 stays sequential (weights
still in flight on the serialized DMA timeline); the last pair hides the
kv flush + repack + M stage + colsum under its q section via per-oc hooks.

Per-core math (T=4096, D=1024, H=16, Dh=64), fp32 PSUM accumulation:
  phase 1 (stream token tiles):
    qT[o,t]   = WqT.T @ xT          (DoubleRow fp8, feature-major)
    phi_qT    = silu(s*qT + s*bq)   (ACT -> fp8, NO +1)
    k[t,e]    = xT.T @ WkT          (DoubleRow fp8, token-major)
    v[t,e]    = xT.T @ WvT          (DoubleRow fp8)
    phi_k     = silu(s*k)           (ACT -> fp8, NO +1)
    v8        = v + bv              (DVE -> fp8)
    kv_h     += v8_h.T @ phi_k_h    (DoubleRow fp8 over 256-token blocks,
                                     on top of the S-init)
  M stage (hidden under the last q section):
    M_h[d,o]  = kv_h.T @ WoT_h      (bf16)  -> m8 fp8 [128,DC,D]
    colsum[o] = ones.T @ m8         (64 tiny matmuls into the freed kv bank)
    bo2       = bo + colsum
  phase 2:
    yT[o,t]   = M.T @ phi_qT + bo2  (DoubleRow fp8) -> bf16 out

Host side: casts x.T / weights to e4m3, computes S exactly, casts y back
to fp32.
"""

import numpy as np
import ml_dtypes

B, T, D = 8, 4096, 1024
H, DH = 16, 64
SCALE = float(DH ** -0.25)
NCORES = 8
P = 128
DC = D // P          # 8 feature chunks
NG = DC // 2         # 4 DoubleRow contraction groups (K=256 each)
TT = 512             # token tile (phase 1)
NTT = T // TT        # 8 token tiles
NSUB = TT // P       # 4 sub-tiles of 128 tokens
NBLK = T // 256      # 16 256-token kv blocks

_BF16 = ml_dtypes.bfloat16
_E4 = ml_dtypes.float8_e4m3

_CACHE = {}


def _split_multi_waits(nc):
    """walrus in this container only encodes ONE sync-wait command per
    instruction. Hoist extra waits onto injected same-engine NOPs placed
    immediately before the instruction (program order on the engine queue
    makes this semantically identical)."""
    import concourse.mybir as mybir

    n_split = 0
    for fn in nc.m.functions:
        for bb in fn.blocks:
            new = []
            changed = False
            for inst in bb.instructions:
                si = inst.sync_info
                waits = list(si.on_wait) if si is not None else []
                if len(waits) > 1:
                    changed = True
                    # satisfy-order heuristic: DMA-queue sems complete last
                    # (the final y transfers), so put them at the END of the
                    # serial NoOp chain — the other NoOps' decode time then
                    # hides under the DMA wait instead of after it
                    waits.sort(
                        key=lambda w: (
                            "DMA" in (w.ant_name or "")
                            or "q" == (w.ant_name or "")[:1]
                        )
                    )
                    for j, w in enumerate(waits[:-1]):
                        nop = mybir.InstNoOp(
                            name=f"{inst.name}-sw{j}", ins=[], outs=[]
                        )
                        nop.engine = inst.engine
                        nop.sync_info = mybir.SyncInfo(
                            on_wait=[w], on_update=[]
                        )
                        new.append(nop)
                        n_split += 1
                    inst.sync_info = mybir.SyncInfo(
                        on_wait=[waits[-1]], on_update=list(si.on_update)
                    )
                new.append(inst)
            if changed:
                bb.instructions = new
    return n_split


def _build_program(debug=False):
    import concourse.bass as bass
    import concourse.mybir as mybir
    from concourse.tile import TileContext, add_dep_helper

    dt = mybir.dt
    AF = mybir.ActivationFunctionType
    DR = mybir.MatmulPerfMode.DoubleRow

    nc = bass.Bass()

    xT_d = nc.dram_tensor("xT8", [D, T], dt.float8e4, kind="ExternalInput")
    wq_d = nc.dram_tensor("wq8", [D, D], dt.float8e4, kind="ExternalInput")
    wk_d = nc.dram_tensor("wk8", [D, D], dt.float8e4, kind="ExternalInput")
    wv_d = nc.dram_tensor("wv8", [D, D], dt.float8e4, kind="ExternalInput")
    wo_d = nc.dram_tensor("wo", [D, D], dt.bfloat16, kind="ExternalInput")
    bqs_d = nc.dram_tensor("bqs", [P, DC], dt.float32, kind="ExternalInput")
    bos_d = nc.dram_tensor("bos", [P, DC], dt.float32, kind="ExternalInput")
    bvb_d = nc.dram_tensor("bvb", [P, D], dt.float32, kind="ExternalInput")
    sv_d = nc.dram_tensor("sv", [DC, P], dt.bfloat16, kind="ExternalInput")
    ev_d = nc.dram_tensor("ev", [DC, 512], dt.bfloat16, kind="ExternalInput")
    yT_d = nc.dram_tensor("yT", [D, T], dt.bfloat16, kind="ExternalOutput")
    if debug:
        phiq_d = nc.dram_tensor("phiq_dump", [P, DC, T], dt.float8e4, kind="ExternalOutput")
        kv_d = nc.dram_tensor("kv_dump", [P, 512], dt.float32, kind="ExternalOutput")
        m_d = nc.dram_tensor("m_dump", [P, DC, D], dt.float8e4, kind="ExternalOutput")
        bo2_d = nc.dram_tensor("bo2_dump", [P, DC], dt.float32, kind="ExternalOutput")

    with TileContext(nc) as tc:
        with (
            tc.tile_pool(name="weights", bufs=1) as wpool,
            tc.tile_pool(name="phiq", bufs=1) as qpool,
            tc.tile_pool(name="msb", bufs=1) as mpool,
            tc.tile_pool(name="xin", bufs=4) as xpool,
            tc.tile_pool(name="kvtiles", bufs=10) as kvpool,
            tc.tile_pool(name="yout", bufs=4) as ypool,
        ):
            # ---- weight / const preload ----
            wq_sb = wpool.tile([P, DC, D], dt.float8e4, tag="wq")
            wk_sb = wpool.tile([P, DC, D], dt.float8e4, tag="wk")
            wv_sb = wpool.tile([P, DC, D], dt.float8e4, tag="wv")
            wo_sb = wpool.tile([P, DC, D], dt.bfloat16, tag="wo")
            bq_sb = wpool.tile([P, DC], dt.float32, tag="bq")
            bos_sb = wpool.tile([P, DC], dt.float32, tag="bos")
            bo2_sb = wpool.tile([P, DC], dt.float32, tag="bo2")
            bv_sb = wpool.tile([P, D], dt.float32, tag="bv")
            sv_sb = wpool.tile([DC, P], dt.bfloat16, tag="sv")
            ev_sb = wpool.tile([DC, 512], dt.bfloat16, tag="ev")
            # The cost model serializes all DMA transfers in arrival order, so
            # issue strictly in first-needed order on the sync queue: tiny
            # consts, wq half, first two x tiles, rest of the weights. Later
            # x tiles go on the gpsimd queue, dep-pinned so they can't race
            # ahead of the weight transfers.
            wq_r = wq_d.rearrange("(c p) o -> p c o", p=P)
            xT_r = xT_d.rearrange("(c p) t -> p c t", p=P)
            # strict first-needed order on one queue: the cost model
            # serializes all DMA transfers, so ordering IS the schedule
            xt_pre = []
            for half in range(2):
                xt0 = xpool.tile([P, DC, TT], dt.float8e4, tag="xt", name=f"xtpre{half}")
                if half == 0:
                    nc.sync.dma_start(xt0[:], xT_r[:, :, 0:TT])
                    # oc0 only needs wq columns 0:128 — tiny first slice so
                    # the first q matmul fires as early as possible
                    nc.sync.dma_start(wq_sb[:, :, 0:128], wq_r[:, :, 0:128])
                else:
                    nc.sync.dma_start(xt0[:], xT_r[:, :, TT : 2 * TT])
                xt_pre.append(xt0)
            nc.sync.dma_start(sv_sb[:], sv_d[:])
            nc.sync.dma_start(bq_sb[:], bqs_d[:])
            nc.sync.dma_start(ev_sb[:], ev_d[:])
            nc.sync.dma_start(wq_sb[:, :, 128:512], wq_r[:, :, 128:512])
            nc.sync.dma_start(wq_sb[:, :, 512:1024], wq_r[:, :, 512:1024])
            wk_r = wk_d.rearrange("(c p) o -> p c o", p=P)
            wv_r = wv_d.rearrange("(c p) o -> p c o", p=P)
            nc.sync.dma_start(wk_sb[:, :, 0:512], wk_r[:, :, 0:512])
            nc.sync.dma_start(wk_sb[:, :, 512:1024], wk_r[:, :, 512:1024])
            nc.sync.dma_start(wv_sb[:, :, 0:512], wv_r[:, :, 0:512])
            nc.sync.dma_start(wv_sb[:, :, 512:1024], wv_r[:, :, 512:1024])
            nc.sync.dma_start(bv_sb[:], bvb_d[:])
            # wo + bos are deferred into the pair loop (pair 3) so the
            # streaming x tiles win the serialized DMA resource first

            phi_q = qpool.tile([P, DC, T], dt.float8e4, tag="phiq")
            m8 = mpool.tile([P, DC, D], dt.float8e4, tag="m8")
            kv_chunks = []
            for c in range(DC):
                kvc = mpool.tile(
                    [P, P], dt.bfloat16, tag=f"kvsb{c}", name=f"kvsb{c}"
                )
                kv_chunks.append(kvc)

            zz = wpool.tile([1, 640], dt.bfloat16, tag="zz")
            nc.vector.memset(zz[:], 0.0)
            ones8 = wpool.tile([P, 1], dt.float8e4, tag="ones8")
            nc.vector.memset(ones8[:], 1.0)
            # kv chunk off-diagonal blocks must be zero (block-diag repack)
            for c in range(DC):
                nc.vector.memset(kv_chunks[c][:], 0.0)

            with tc.tile_pool(name="ps_kv", bufs=1, space="PSUM") as pkv_pool:
                # two kv banks: heads 0-7 in A, 8-15 in B, every head's
                # [64,64] kvT block at PSUM rows 0:64 so ALL kv accumulation
                # matmuls are DoubleRow-legal (dst base partition 0)
                kv_psA = pkv_pool.tile([P, 512], dt.float32, tag="kvaccA")
                kv_psB = pkv_pool.tile([P, 512], dt.float32, tag="kvaccB")
                # warmup matmuls filling the startup DMA shadow: they keep the
                # PE p-state warm so the first real matmuls run at full clock
                # the moment weights land. The kv bank content they write is
                # dead — the S-init matmul below re-initializes it.
                nc.tensor.matmul(
                    kv_psA[:], lhsT=zz[:1, :P], rhs=zz[:1, P : P + 512],
                    start=True, stop=True, skip_group_check=True,
                )
                for w in range(38):
                    nc.tensor.matmul(
                        kv_psA[:, 0:128], lhsT=zz[:1, :P], rhs=zz[:1, P : P + 128],
                        start=True, stop=True, skip_group_check=True,
                    )

                kv_pend = [None]

                def _emit_kv(pending, last):
                    # one DoubleRow matmul per head per 256-token block, all
                    # at dst base partition 0 (ISA requirement for DR)
                    phik_p, vsb_p = pending
                    for h in range(H):
                        bank = kv_psA if h < 8 else kv_psB
                        c0 = (h % 8) * 64
                        nc.tensor.matmul(
                            bank[0:64, c0 : c0 + 64],
                            lhsT=vsb_p[:, :, h * 64 : (h + 1) * 64],
                            rhs=phik_p[:, :, h * 64 : (h + 1) * 64],
                            start=False,
                            stop=last and h == H - 1,
                            perf_mode=DR,
                            skip_group_check=True,
                        )

                with tc.tile_pool(name="ps_q", bufs=6, space="PSUM") as pq_pool:
                  if True:
                      pkvp_pool = pq_pool
                      # ---- q projection (feature-major out, DoubleRow).
                      # first_split runs tile A before tile B (pair 0: B's DMA
                      # still in flight), bridging with PE warmup dummies.
                      def _q_section(pair, xts, first_split, post_oc=None):
                          def _drain(oc, half, psx):
                              tt = pair * 2 + half
                              pq_slice = phi_q[:, oc, tt * TT : (tt + 1) * TT]
                              nc.scalar.activation(
                                  pq_slice, psx[:], AF.Silu,
                                  bias=bq_sb[:, oc : oc + 1], scale=SCALE,
                              )

                          for oc in range(DC):
                              psA = pq_pool.tile([P, TT], dt.float32, tag="ps")
                              psB = pq_pool.tile([P, TT], dt.float32, tag="ps")
                              if first_split and oc == 0:
                                  last_a = None
                                  for g in range(NG):
                                      last_a = nc.tensor.matmul(
                                          psA[:],
                                          lhsT=wq_sb[:, 2 * g : 2 * g + 2, oc * P : (oc + 1) * P],
                                          rhs=xts[0][:, 2 * g : 2 * g + 2, :],
                                          start=(g == 0),
                                          stop=(g == NG - 1),
                                          perf_mode=DR,
                                      )
                                  # bridge the B-tile DMA wait with warmup
                                  # zero-rewrites of the (still dead) kv bank;
                                  # dep-pinned after the A matmuls so the
                                  # scheduler cannot hoist them earlier
                                  for w in range(14):
                                      dmy = nc.tensor.matmul(
                                          kv_psA[:, 0:128],
                                          lhsT=zz[:1, :P],
                                          rhs=zz[:1, P : P + 128],
                                          start=True, stop=True,
                                          skip_group_check=True,
                                      )
                                      add_dep_helper(
                                          dmy.ins, last_a.ins, sync=False,
                                          reason="bridge dummies after A matmuls",
                                      )
                                  # S-init: bank[r, c] = S[hbase + (c//64)*64 + r]
                                  # (the phi_k "+1" fold; kv blocks hold
                                  # kv_h TRANSPOSED so S varies along rows).
                                  # Must be the LAST start=True write to each
                                  # kv bank.
                                  for bank, c0_ in ((kv_psA, 0), (kv_psB, 64)):
                                      dmy = nc.tensor.matmul(
                                          bank[0:64, :],
                                          lhsT=sv_sb[:, c0_ : c0_ + 64],
                                          rhs=ev_sb[:, :],
                                          start=True, stop=True,
                                          skip_group_check=True,
                                      )
                                      add_dep_helper(
                                          dmy.ins, last_a.ins, sync=False,
                                          reason="S-init after A matmuls",
                                      )
                                  for g in range(NG):
                                      nc.tensor.matmul(
                                          psB[:],
                                          lhsT=wq_sb[:, 2 * g : 2 * g + 2, oc * P : (oc + 1) * P],
                                          rhs=xts[1][:, 2 * g : 2 * g + 2, :],
                                          start=(g == 0),
                                          stop=(g == NG - 1),
                                          perf_mode=DR,
                                      )
                              else:
                                  for g in range(NG):
                                      nc.tensor.matmul(
                                          psA[:],
                                          lhsT=wq_sb[:, 2 * g : 2 * g + 2, oc * P : (oc + 1) * P],
                                          rhs=xts[0][:, 2 * g : 2 * g + 2, :],
                                          start=(g == 0),
                                          stop=(g == NG - 1),
                                          perf_mode=DR,
                                      )
                                      nc.tensor.matmul(
                                          psB[:],
                                          lhsT=wq_sb[:, 2 * g : 2 * g + 2, oc * P : (oc + 1) * P],
                                          rhs=xts[1][:, 2 * g : 2 * g + 2, :],
                                          start=(g == 0),
                                          stop=(g == NG - 1),
                                          perf_mode=DR,
                                      )
                              _drain(oc, 0, psA)
                              _drain(oc, 1, psB)
                              if post_oc is not None and oc in post_oc:
                                  post_oc[oc]()

                      # ---- k,v projections (token-major, DoubleRow) + kv
                      # accumulation over 256-token blocks. kv matmuls for a
                      # block are emitted one block LATE so PE never waits on
                      # the silu/+bv drains. ----
                      blk_state = {}

                      def _kv_sub(pair, xts, half, sub):
                          tt = pair * 2 + half
                          xt = xts[half]
                          blk = (tt * NSUB + sub) // 2
                          i = sub % 2
                          if i == 0:
                              phik_b = kvpool.tile([P, 2, D], dt.float8e4, tag="phik")
                              v_b = kvpool.tile([P, 2, D], dt.float8e4, tag="vsb")
                              blk_state[blk] = (phik_b, v_b)
                          else:
                              phik_b, v_b = blk_state[blk]
                          # 4 one-bank psum tiles per sub so each half-drain
                          # releases its bank early
                          pk0 = pkvp_pool.tile([P, 512], dt.float32, tag="ps")
                          pk1 = pkvp_pool.tile([P, 512], dt.float32, tag="ps")
                          pv0 = pkvp_pool.tile([P, 512], dt.float32, tag="ps")
                          pv1 = pkvp_pool.tile([P, 512], dt.float32, tag="ps")
                          kvps = (pk0, pk1, pv0, pv1)
                          xs = xt[:, :, sub * P : (sub + 1) * P]
                          for g in range(NG):
                              for n, ps in enumerate(kvps):
                                  w_sb = wk_sb if n < 2 else wv_sb
                                  col = (n % 2) * 512
                                  nc.tensor.matmul(
                                      ps[:],
                                      lhsT=xs[:, 2 * g : 2 * g + 2, :],
                                      rhs=w_sb[:, 2 * g : 2 * g + 2, col : col + 512],
                                      start=(g == 0),
                                      stop=(g == NG - 1),
                                      perf_mode=DR,
                                  )
                          nc.scalar.activation(
                              phik_b[:, i, 0:512], pk0[:], AF.Silu, scale=SCALE
                          )
                          nc.vector.tensor_add(
                              v_b[:, i, 0:512], pv0[:], bv_sb[:, 0:512]
                          )
                          nc.scalar.activation(
                              phik_b[:, i, 512:1024], pk1[:], AF.Silu, scale=SCALE
                          )
                          nc.vector.tensor_add(
                              v_b[:, i, 512:1024], pv1[:], bv_sb[:, 512:1024]
                          )
                          if i == 1:
                              if kv_pend[0] is not None:
                                  _emit_kv(kv_pend[0], False)
                              kv_pend[0] = blk_state.pop(blk)

                      def _kvproj_section(pair, xts):
                          for half in range(2):
                              for sub in range(NSUB):
                                  _kv_sub(pair, xts, half, sub)

                      for pair in range(NTT // 2):
                          if pair == 0:
                              xts = xt_pre
                          else:
                              xts = []
                              for half in range(2):
                                  tt = pair * 2 + half
                                  xt = xpool.tile([P, DC, TT], dt.float8e4, tag="xt")
                                  nc.sync.dma_start(
                                      xt[:], xT_r[:, :, tt * TT : (tt + 1) * TT]
                                  )
                                  xts.append(xt)
                              if pair == NTT // 2 - 1:
                                  nc.sync.dma_start(
                                      wo_sb[:],
                                      wo_d.rearrange("(c p) o -> p c o", p=P),
                                  )
                                  nc.sync.dma_start(bos_sb[:], bos_d[:])

                          if pair == NTT // 2 - 1:
                              # last pair: kvproj first, then q. The kv flush,
                              # repack copies, M matmuls, m8 drains, colsum and
                              # bo2 are spread across the q chunk boundaries,
                              # hidden under the q matmul stream. (Not
                              # interleaved: kv must finish before the flush.)
                              _kvproj_section(pair, xts)

                              def _hook_flush():
                                  _emit_kv(kv_pend[0], True)
                                  kv_pend[0] = None

                              def _repack(c):
                                  # repack chunk c, alternating engines
                                  if c % 2 == 0:
                                      use_dve = True
                                  else:
                                      use_dve = False
                                  for hh, rr in ((2 * c, 0), (2 * c + 1, 64)):
                                      bank = kv_psA if hh < 8 else kv_psB
                                      src_c = (hh % 8) * 64
                                      dst = kv_chunks[c][rr : rr + 64, rr : rr + 64]
                                      if use_dve:
                                          nc.vector.tensor_copy(
                                              out=dst,
                                              in_=bank[0:64, src_c : src_c + 64],
                                          )
                                      else:
                                          nc.scalar.copy(
                                              out=dst,
                                              in_=bank[0:64, src_c : src_c + 64],
                                          )

                              def _mk_hook_m(c):
                                  def _hook():
                                      if c == 0:
                                          for cc in range(DC):
                                              _repack(cc)
                                      for n in range(2):
                                          pm = pkvp_pool.tile(
                                              [P, 512], dt.float32, tag="ps"
                                          )
                                          nc.tensor.matmul(
                                              pm[:],
                                              lhsT=kv_chunks[c][:],
                                              rhs=wo_sb[:, c, n * 512 : (n + 1) * 512],
                                              start=True,
                                              stop=True,
                                          )
                                          # DVE-only drains: ACT is the
                                          # scarce engine in the last-pair
                                          # q section
                                          dst = m8[:, c, n * 512 : (n + 1) * 512]
                                          nc.vector.tensor_copy(
                                              out=dst, in_=pm[:]
                                          )
                                  return _hook

                              def _colsum_chunks(cs, first, last):
                                  # colsum[o] = sum_f m8[f, o] accumulated in
                                  # the freed kv bank, cols 0:DC, one column
                                  # per o-chunk — spread incrementally so each
                                  # piece waits only on its own m8 chunks.
                                  if first:
                                      nc.tensor.matmul(
                                          kv_psA[:, 0:DC], lhsT=zz[:1, :P],
                                          rhs=zz[:1, P : P + DC],
                                          start=True, stop=True,
                                          skip_group_check=True,
                                      )
                                  for c in cs:
                                      for oc2 in range(DC):
                                          nc.tensor.matmul(
                                              kv_psA[:, oc2 : oc2 + 1],
                                              lhsT=m8[:, c, oc2 * P : (oc2 + 1) * P],
                                              rhs=ones8[:, 0:1],
                                              start=False,
                                              stop=(last and c == cs[-1] and oc2 == DC - 1),
                                              skip_group_check=True,
                                          )
                                  if last:
                                      nc.vector.tensor_add(
                                          bo2_sb[:], kv_psA[:, 0:DC], bos_sb[:]
                                      )

                              # hook layout: flush+repack+M(c0) at oc0, then
                              # one M chunk per oc, with each chunk's colsum
                              # two ocs behind its m8 drain; the last two
                              # colsum pieces + bo2 land right after the
                              # section (phase 2's first drain ~1us in).
                              def _mk_hook(oc):
                                  def _hook():
                                      if oc == 0:
                                          _hook_flush()
                                      _mk_hook_m(oc)()
                                      if oc >= 2:
                                          _colsum_chunks(
                                              [oc - 2], first=(oc == 2),
                                              last=False,
                                          )
                                  return _hook

                              hooks = {oc: _mk_hook(oc) for oc in range(DC)}
                              _q_section(pair, xts, False, hooks)
                              _colsum_chunks([DC - 2], first=False, last=False)
                              _colsum_chunks([DC - 1], first=False, last=True)
                          else:
                              # interleave kv subs between q oc boundaries so
                              # ACT drain demand never bursts above the PE
                              # rate. Pair 0 front-loads 4 q ocs (the kv
                              # weights are still in flight on the serialized
                              # DMA timeline), then 2 subs per oc.
                              def _mk_kv_hook(subs, pair=pair, xts=xts):
                                  def _h():
                                      for s in subs:
                                          _kv_sub(pair, xts, s // NSUB, s % NSUB)
                                  return _h

                              if pair == 0:
                                  hooks = {
                                      3 + j: _mk_kv_hook([2 * j, 2 * j + 1])
                                      for j in range(4)
                                  }
                              else:
                                  hooks = {
                                      i: _mk_kv_hook([i]) for i in range(DC)
                                  }
                              _q_section(pair, xts, pair == 0, hooks)

                      if kv_pend[0] is not None:
                          _emit_kv(kv_pend[0], True)
                          kv_pend[0] = None

                if debug:
                    kvf = mpool.tile([P, 512], dt.float32, tag="kvdump")
                    nc.vector.tensor_copy(out=kvf[:], in_=kv_psA[:])
                    nc.sync.dma_start(kv_d[:], kvf[:])
                    bo2f = mpool.tile([P, DC], dt.float32, tag="bo2dump")
                    nc.vector.tensor_copy(out=bo2f[:], in_=bo2_sb[:])
                    nc.sync.dma_start(bo2_d[:], bo2f[:])
            # ---- phase 2: yT = M.T @ phi_q + bo2 (DoubleRow) ----
            # Per oc: 4 qb blocks of [128,1024] psum; drains split
            # ACT/DVE per 512 half into a [P, T] bf16 staging tile,
            # DMA'd out in 2048-token halves on sync/gpsimd queues.
            # (kv pool closed above -> 8 banks free for bufs=4.)
            with tc.tile_pool(name="ps_y", bufs=4, space="PSUM") as py_pool:
                for oc in range(DC):
                    ys = ypool.tile([P, T], dt.bfloat16, tag="ys")
                    for qb in range(4):
                        py = py_pool.tile([P, 1024], dt.float32, tag="py")
                        for i in range(2):
                            for g in range(NG):
                                nc.tensor.matmul(
                                    py[:, i * 512 : (i + 1) * 512],
                                    lhsT=m8[:, 2 * g : 2 * g + 2, oc * P : (oc + 1) * P],
                                    rhs=phi_q[
                                        :, 2 * g : 2 * g + 2,
                                        qb * 1024 + i * 512 : qb * 1024 + (i + 1) * 512,
                                    ],
                                    start=(g == 0),
                                    stop=(g == NG - 1),
                                    perf_mode=DR,
                                )
                        # one full-width drain per qb, alternating engines:
                        # halves the instruction+semaphore count vs split
                        # drains, keeping both engines under the PE rate.
                        # Very last block: two half drains on both engines so
                        # the kernel tail is one small piece.
                        ys_sl = ys[:, qb * 1024 : (qb + 1) * 1024]
                        if oc == DC - 1 and qb == 3:
                            nc.scalar.activation(
                                ys[:, qb * 1024 : qb * 1024 + 512],
                                py[:, 0:512], AF.Identity,
                                bias=bo2_sb[:, oc : oc + 1], scale=1.0,
                            )
                            nc.vector.tensor_scalar_add(
                                ys[:, qb * 1024 + 512 : (qb + 1) * 1024],
                                py[:, 512:1024], bo2_sb[:, oc : oc + 1],
                            )
                        elif qb % 2 == 1:
                            nc.scalar.activation(
                                ys_sl, py[:], AF.Identity,
                                bias=bo2_sb[:, oc : oc + 1], scale=1.0,
                            )
                        else:
                            nc.vector.tensor_scalar_add(
                                ys_sl, py[:], bo2_sb[:, oc : oc + 1],
                            )
                        # per-qb 1024-token DMAs spread evenly over both
                        # queues so no end-of-kernel burst; the final qb goes
                        # out as two 512 halves so the last transfer is small
                        if oc == DC - 1 and qb == 3:
                            nc.sync.dma_start(
                                yT_d[oc * P : (oc + 1) * P, 3072:3584],
                                ys[:, 3072:3584],
                            )
                            nc.gpsimd.dma_start(
                                yT_d[oc * P : (oc + 1) * P, 3584:4096],
                                ys[:, 3584:4096],
                            )
                        else:
                            q_eng = nc.sync if qb % 2 == 0 else nc.gpsimd
                            q_eng.dma_start(
                                yT_d[oc * P : (oc + 1) * P,
                                     qb * 1024 : (qb + 1) * 1024],
                                ys[:, qb * 1024 : (qb + 1) * 1024],
                            )

            if debug:
                nc.sync.dma_start(phiq_d[:], phi_q[:])
                nc.sync.dma_start(m_d[:], m8[:])
    _split_multi_waits(nc)
    return nc


def _get_program(debug=False):
    key = ("nc", debug)
    if key not in _CACHE:
        _CACHE[key] = _build_program(debug)
    return _CACHE[key]


def _prep_shared(Wq, bq, Wk, Wv, bv, Wo, bo):
    shared = {
        "wq8": np.ascontiguousarray(Wq.T).astype(_E4),
        "wk8": np.ascontiguousarray(Wk.T).astype(_E4),
        "wv8": np.ascontiguousarray(Wv.T).astype(_E4),
        "wo": np.ascontiguousarray(Wo.T).astype(_BF16),
        "bqs": np.ascontiguousarray(
            (SCALE * bq).astype(np.float32).reshape(DC, P).T
        ),
        "bos": np.ascontiguousarray(bo.astype(np.float32).reshape(DC, P).T),
        "bvb": np.ascontiguousarray(
            np.broadcast_to(bv.astype(np.float32), (P, D))
        ),
        "ev": np.ascontiguousarray(
            np.repeat(np.eye(DC, dtype=np.float32), 64, axis=1)
        ).astype(_BF16),
    }
    return shared


def _pack_sv(S):
    # bank[r, c] inits to S[hbase*64 + (c//64)*64 + r]: per bank a rank-8
    # matmul with U = S-slice.reshape(8, 64) stationary (column-packed so
    # both banks' U slices sit at base partition 0) and the 64-wide column
    # indicator moving (built in _prep_shared)
    s2 = np.concatenate(
        [S[:512].reshape(DC, 64), S[512:].reshape(DC, 64)], axis=1
    )
    return np.ascontiguousarray(s2).astype(_BF16)


def _run(in_maps, trace=False, debug=False, **kw):
    from concourse.bass_utils import run_bass_kernel_spmd

    nc = _get_program(debug)
    return run_bass_kernel_spmd(nc, in_maps, list(range(NCORES)), trace=trace, **kw)


def kernel(x, Wq, bq, Wk, Wv, bv, Wo, bo):
    x = np.asarray(x, dtype=np.float32)
    assert x.shape == (B, T, D), x.shape
    Wv32 = np.asarray(Wv, np.float32)
    bv32 = np.asarray(bv, np.float32)
    shared = _prep_shared(
        np.asarray(Wq, np.float32), np.asarray(bq, np.float32),
        np.asarray(Wk, np.float32), Wv32,
        bv32, np.asarray(Wo, np.float32),
        np.asarray(bo, np.float32),
    )
    xsum = x.sum(axis=1)  # [B, D]
    in_maps = []
    for b in range(B):
        m = dict(shared)
        m["xT8"] = np.ascontiguousarray(x[b].T).astype(_E4)
        S = xsum[b] @ Wv32.T + T * bv32
        m["sv"] = _pack_sv(S)
        in_maps.append(m)

    res = _run(in_maps)
    out = np.empty((B, T, D), np.float32)
    for b in range(B):
        out[b] = res.results[b]["yT"].T.astype(np.float32)
    return out
